# revision 18
# baseline (speedup 1.0000x reference)
"""Causal self-attention kernel for 8 Trainium2 NeuronCores.

Problem: B=4, T=2048, C=1024, NH=16, HD=64 (fp32 in/out).
Sharding: 8 cores = 4 batches x 2 head-groups (8 heads each).
Each core computes qkv projection + causal attention + its partial c_proj
for (batch b, heads hg*8..hg*8+7); host sums the two head-group partials.

v3 changes vs v2:
  * Software-pipelined attention inner loop: y~ matmuls trail the S
    matmuls by two iterations so the in-order PE queue never waits on
    the ScalarE exp of the same iteration.
  * Projection / c_proj phases are split into ~1us units and pumped
    one-per-attention-iteration from a FIFO, filling the PE gap left
    by the exp-bound attention stream (PE 640ns/iter vs ACT 985ns).

v2 (kept): all PE operands bf16 (fp32r measured ~2 cyc/row warm vs bf16
1 cyc/row); causality at 128-key granularity with windowed S/exp/y and a
single shared [128,128] lower-triangle diagonal mask.

Dataflow per core: x --PE-transpose--> x^T; q^T,k^T in [feat, T] layout
(2 heads x 64 dims = 128 partitions); v in [T, feat] with a ones column
per head (softmax denominator rides row 64 of the y~ PSUM).
S^T[k,q] = k^T.T @ q^T via two row-packed K=64 matmuls; exp on ScalarE;
y~^T = v_aug.T @ P^T on PE; normalize via DVE reciprocal + GPSIMD
broadcast; c_proj from y^T tiles.
"""

import collections
import math

import ml_dtypes
import numpy as np

import concourse.bass as bass
import concourse.mybir as mybir
import concourse.tile as tile
from concourse import bacc
from concourse.bass_utils import run_bass_kernel_spmd

F32R = mybir.dt.float32r
F32 = mybir.dt.float32
BF16 = mybir.dt.bfloat16
EXP = mybir.ActivationFunctionType.Exp
BF = ml_dtypes.bfloat16

B, T, C = 4, 2048, 1024
NH, HD = 16, 64
NHL = 8            # heads per core
PAIRS = 4          # head pairs per core
CH = 512           # q-chunk width
NCH = T // CH      # 4 q-chunks
KT = C // 128      # 8 contraction tiles over C
NTT = T // 128     # 16 T-tiles
SCALE = 1.0 / math.sqrt(HD)
NEG = -1.0e30


def build_nc():
    nc = bacc.Bacc("TRN2", target_bir_lowering=False)

    x_d = nc.dram_tensor("x_l", [T, C], BF16, kind="ExternalInput")
    wqk_d = nc.dram_tensor("w_qk", [1024, 1024], BF16, kind="ExternalInput")
    wv_d = nc.dram_tensor("w_v", [128, 4096], BF16, kind="ExternalInput")
    wp_d = nc.dram_tensor("w_p", [128, 4096], BF16, kind="ExternalInput")
    bqk_d = nc.dram_tensor("b_qk", [128, 8], F32, kind="ExternalInput")
    bv_d = nc.dram_tensor("b_v", [512], F32, kind="ExternalInput")
    bo_d = nc.dram_tensor("b_o", [C], F32, kind="ExternalInput")
    id_d = nc.dram_tensor("ident", [128, 128], BF16, kind="ExternalInput")
    mask_d = nc.dram_tensor("masks", [128, 128], BF16, kind="ExternalInput")
    out_d = nc.dram_tensor("out_p", [T, C], BF16, kind="ExternalOutput")

    with tile.TileContext(nc) as tc:
        with tc.tile_pool(name="cp", bufs=1) as cp, \
             tc.tile_pool(name="wk", bufs=1) as wk, \
             tc.tile_pool(name="ps", bufs=1, space="PSUM") as ps:
            # ---- constants (ident first: first transposes need only it) ----
            ident = cp.tile([128, 128], BF16, name="ident")
            nc.scalar.dma_start(ident, id_d.ap())
            bqk = cp.tile([128, 8], F32, name="bqk")
            nc.scalar.dma_start(bqk, bqk_d.ap())
            # first q/k weight slices ahead of the scalar-queue x half so
            # proj_qk(0) can start right after the first transposes
            wq00 = cp.tile([128, 8, 128], BF16, name="wq00")
            nc.scalar.dma_start(
                wq00, wqk_d.ap()[0:128, :].rearrange("p (a j) -> p a j", j=128))
            wq04 = cp.tile([128, 8, 128], BF16, name="wq04")
            nc.scalar.dma_start(
                wq04, wqk_d.ap()[512:640, :].rearrange("p (a j) -> p a j", j=128))
            # prefetch chunk-0 x tiles split across two DMA queues
            xin0 = []
            for t4 in range(4):
                xi = wk.tile([128, C], BF16, tag="xin", bufs=4,
                             name=f"xin{t4}")
                eng = nc.sync if t4 < 2 else nc.scalar
                eng.dma_start(xi, x_d.ap()[t4 * 128:(t4 + 1) * 128, :])
                xin0.append(xi)
            wv = cp.tile([128, 8, 512], BF16, name="wv")
            bv_row = cp.tile([1, 512], F32, name="bv_row")
            bv_rep = cp.tile([128, 512], F32, name="bv_rep")
            masks = cp.tile([128, 128], BF16, name="masks")
            bo_row = cp.tile([1, 1024], F32, name="bo_row")
            bo_rep = cp.tile([128, 1024], F32, name="bo_rep")
            wp = cp.tile([128, 4, 2, 512], BF16, name="wp")
            consts_loaded = set()

            def load_v_consts():
                if "v" in consts_loaded:
                    return
                consts_loaded.add("v")
                nc.sync.dma_start(
                    wv, wv_d.ap().rearrange("p (a n) -> p a n", n=512))
                nc.sync.dma_start(
                    bv_row, bv_d.ap().rearrange("(a n) -> a n", a=1))
                nc.gpsimd.partition_broadcast(bv_rep, bv_row)

            def load_a_consts():
                if "a" in consts_loaded:
                    return
                consts_loaded.add("a")
                nc.scalar.dma_start(masks, mask_d.ap())

            def load_c_consts():
                if "c" in consts_loaded:
                    return
                consts_loaded.add("c")
                nc.sync.dma_start(
                    wp, wp_d.ap().rearrange("p (a b n) -> p a b n",
                                            a=4, b=2, n=512))
                nc.sync.dma_start(
                    bo_row, bo_d.ap().rearrange("(a n) -> a n", a=1))
                nc.gpsimd.partition_broadcast(bo_rep, bo_row)

            # ---- persistent activations ----
            kT = [cp.tile([128, T], BF16, name=f"kT{p}") for p in range(PAIRS)]
            vt = cp.tile([128, NTT, 8 * 65], BF16, name="vt")

            qT = {}   # (pair, chunk) -> [128, 512] tile
            yT = {}   # (pair, chunk) -> [128, 512] tile
            XTS = {}  # chunk -> list of x^T tiles

            # ---------- filler unit machinery ----------
            filler = collections.deque()   # of (group, fn)

            def unit(group, fn):
                filler.append((group, fn))

            def pump(n=1):
                for _ in range(n):
                    if filler:
                        _, fn = filler.popleft()
                        fn()

            def require(*groups):
                gs = set(groups)
                while any(g in gs for g, _ in filler):
                    _, fn = filler.popleft()
                    fn()

            def flush_all():
                while filler:
                    _, fn = filler.popleft()
                    fn()

            # ---------- proj units ----------
            def init_xt(c, xin_pre=None):
                st = {'xts': {}}
                XTS[c] = st
                if xin_pre is not None:
                    st['xin'] = xin_pre
                return st

            def enq_xt_load(c):
                def load(c=c):
                    xin = []
                    for t4 in range(4):
                        tt = c * 4 + t4
                        xi = wk.tile([128, C], BF16, tag="xin", bufs=4,
                                     name=f"xin{tt}")
                        nc.sync.dma_start(
                            xi, x_d.ap()[tt * 128:(tt + 1) * 128, :])
                        xin.append(xi)
                    XTS[c]['xin'] = xin

                unit(f"xt{c}", load)

            def enq_xt_tr(c, kc):
                def tr(c=c, kc=kc):
                    st = XTS[c]
                    xin = st['xin']
                    xt_ps = ps.tile([128, 512], BF16, tag="pj", bufs=2,
                                    name=f"xtps{c}_{kc}")
                    for t4 in range(4):
                        nc.tensor.transpose(
                            xt_ps[:, t4 * 128:(t4 + 1) * 128],
                            xin[t4][:, kc * 128:(kc + 1) * 128], ident)
                    xt = wk.tile([128, 512], BF16, tag="xt", bufs=16,
                                 name=f"xt{c}_{kc}")
                    nc.vector.tensor_copy(xt, xt_ps)
                    st['xts'][kc] = xt

                unit(f"xt{c}", tr)

            wq_tiles = {}
            wq_dma_enqueued = set()

            def enq_wq_dma(c, f):
                # separate DMA unit so the weight load runs ~2 units ahead
                # of the matmuls that consume it
                if (c, f) in wq_dma_enqueued:
                    return
                wq_dma_enqueued.add((c, f))

                def run(c=c, f=f):
                    wq = wk.tile([128, 8, 128], BF16, tag="wqk", bufs=4,
                                 name=f"wq{c}_{f}")
                    nc.scalar.dma_start(
                        wq, wqk_d.ap()[f * 128:(f + 1) * 128, :]
                        .rearrange("p (a j) -> p a j", j=128))
                    wq_tiles[(c, f)] = wq

                unit(f"wqdma{c}_{f}", run)

            def enq_qk_f(c, f):
                g = f"qk{c}_{f}"

                def run(c=c, f=f):
                    xts = XTS[c]['xts']
                    wq = wq_tiles.pop((c, f))
                    qk_ps = ps.tile([128, 512], F32, tag="pj", bufs=2,
                                    name=f"qkps{c}_{f}")
                    for kt in range(KT):
                        nc.tensor.matmul(qk_ps, wq[:, kt, :], xts[kt],
                                         start=(kt == 0), stop=(kt == KT - 1))
                    if f < 4:
                        qt = wk.tile([128, 512], BF16, tag="qT", bufs=8,
                                     name=f"qT{f}_{c}")
                        nc.vector.tensor_scalar_add(qt, qk_ps, bqk[:, f:f + 1])
                        qT[(f, c)] = qt
                    else:
                        nc.vector.tensor_scalar_add(
                            kT[f - 4][:, c * CH:(c + 1) * CH], qk_ps,
                            bqk[:, f:f + 1])

                unit(g, run)

            def enq_v(c):
                g = f"v{c}"

                def run_t4(t4, c=c):
                    load_v_consts()
                    xts = XTS[c]['xts']
                    tt = c * 4 + t4
                    v_ps = ps.tile([128, 512], F32, tag="pj", bufs=2,
                                   name=f"vps{tt}")
                    for kt in range(KT):
                        nc.tensor.matmul(
                            v_ps, xts[kt][:, t4 * 128:(t4 + 1) * 128],
                            wv[:, kt, :],
                            start=(kt == 0), stop=(kt == KT - 1))
                    vslice = vt[:, tt, :].rearrange("p (h e) -> p h e", e=65)
                    nc.gpsimd.memset(vslice[:, :, 64:65], 1.0)
                    nc.vector.tensor_add(
                        vslice[:, :, 0:64],
                        v_ps.rearrange("p (h e) -> p h e", e=64),
                        bv_rep.rearrange("p (h e) -> p h e", e=64))

                for t4 in range(4):
                    unit(g, lambda t4=t4: run_t4(t4))

            def enq_cproj(c, half):
                g = f"cp{c}_{half}"

                def run(t4, oc, c=c, half=half):
                    load_c_consts()
                    tt = c * 4 + t4
                    o_ps = ps.tile([128, 512], F32, tag="pj", bufs=2,
                                   name=f"ops{tt}_{oc}")
                    for p in range(PAIRS):
                        nc.tensor.matmul(
                            o_ps,
                            yT[(p, c)][:, t4 * 128:(t4 + 1) * 128],
                            wp[:, p, oc, :],
                            start=(p == 0), stop=(p == PAIRS - 1))
                    ot = wk.tile([128, 512], BF16, tag="o", bufs=2,
                                 name=f"o{tt}_{oc}")
                    nc.vector.tensor_add(
                        ot, o_ps, bo_rep[:, oc * 512:(oc + 1) * 512])
                    nc.sync.dma_start(
                        out_d.ap()[tt * 128:(tt + 1) * 128,
                                   oc * 512:(oc + 1) * 512], ot)

                for t4 in range(2 * half, 2 * half + 2):
                    for oc in range(2):
                        unit(g, lambda t4=t4, oc=oc: run(t4, oc))

            # ---------- attention (software-pipelined, pumps filler) ----------
            def attn_pair(c, p):
                ctx = nc.named_scope(f"at{c}_{p}"); ctx.__enter__()
                load_a_consts()
                nkt = 4 * (c + 1)
                yA = ps.tile([65, 512], F32, tag="y", bufs=2,
                             name=f"yA{p}_{c}")
                yB = ps.tile([65, 512], F32, tag="y", bufs=2,
                             name=f"yB{p}_{c}")
                qtc = qT.pop((p, c))
                pend = collections.deque()  # (kt, w0, pt) awaiting y emission

                def emit_y(kt, w0, pt):
                    nc.tensor.matmul(
                        yA[:, w0:512],
                        vt[:, kt, (2 * p) * 65:(2 * p) * 65 + 65],
                        pt[:, w0:512],
                        start=(kt == 0), stop=(kt == nkt - 1))
                    nc.tensor.matmul(
                        yB[:, w0:512],
                        vt[:, kt, (2 * p + 1) * 65:(2 * p + 1) * 65 + 65],
                        pt[:, 512 + w0:1024],
                        start=(kt == 0), stop=(kt == nkt - 1))

                for kt in range(nkt):
                    s_ps = ps.tile([128, 1024], F32, tag="s", bufs=2,
                                   name=f"s{p}_{c}_{kt}")
                    d = kt * 128 - c * CH
                    diag = d >= 0
                    w0 = d if diag else 0   # valid q-window is [w0, 512)
                    ksl = kT[p][:, kt * 128:(kt + 1) * 128]
                    nc.tensor.matmul(s_ps[:, w0:512], ksl[0:64, :],
                                     qtc[0:64, w0:512], start=True,
                                     stop=not diag, tile_position=(0, 0))
                    nc.tensor.matmul(s_ps[:, 512 + w0:1024], ksl[64:128, :],
                                     qtc[64:128, w0:512], start=True,
                                     stop=not diag,
                                     tile_position=(64, 0))
                    if diag:
                        # lower-triangle -inf on the [128,128] diagonal block
                        nc.tensor.matmul(s_ps[:, w0:w0 + 128], ident, masks,
                                         start=False, stop=True)
                        nc.tensor.matmul(s_ps[:, 512 + w0:512 + w0 + 128],
                                         ident, masks,
                                         start=False, stop=True)
                    pt = wk.tile([128, 1024], BF16, tag="P", bufs=4,
                                 name=f"P{p}_{c}_{kt}")
                    if w0:
                        s_in = s_ps.rearrange("p (h q) -> p h q",
                                              h=2)[:, :, w0:512]
                        p_out = pt.rearrange("p (h q) -> p h q",
                                             h=2)[:, :, w0:512]
                    else:
                        s_in, p_out = s_ps, pt
                    nc.scalar.activation(p_out, s_in, EXP, scale=SCALE)
                    pend.append((kt, w0, pt))
                    if len(pend) > 2:
                        emit_y(*pend.popleft())
                    pump()
                while pend:
                    emit_y(*pend.popleft())
                yt = wk.tile([128, 512], BF16, tag="yT", bufs=12,
                             name=f"yT{p}_{c}")
                rrs = []
                for h, yps in ((0, yA), (1, yB)):
                    drow = wk.tile([1, 512], F32, tag="rc", bufs=4,
                                   name=f"dr{p}_{c}_{h}")
                    nc.vector.tensor_copy(drow, yps[64:65, :])
                    rc = wk.tile([1, 512], F32, tag="rc", bufs=4,
                                 name=f"rc{p}_{c}_{h}")
                    nc.vector.reciprocal_approx_fast(rc, drow)
                    rr = wk.tile([64, 512], F32, tag="rr", bufs=2,
                                 name=f"rr{p}_{c}_{h}")
                    nc.gpsimd.partition_broadcast(rr, rc)
                    rrs.append(rr)
                for h, yps in ((0, yA), (1, yB)):
                    nc.vector.tensor_mul(yt[h * 64:(h + 1) * 64, :],
                                         yps[0:64, :], rrs[h])
                yT[(p, c)] = yt
                ctx.__exit__(None, None, None)

            # ---------- schedule ----------
            def enq_block_fillers(cq, cn, cps):
                """Fillers pumped during attn(cq, *): remaining qk f-units of
                chunk cq (with wq DMA prefetched 2 units ahead), x^T prep of
                chunk cn, and c_proj units of earlier chunks."""
                if cn is not None:
                    enq_xt_load(cn)
                if cq < 3:
                    # prefetch next block's first q/k weight loads early
                    enq_wq_dma(cq + 1, 0); enq_wq_dma(cq + 1, 4)
                for i, (fq, fk) in enumerate(((1, 5), (2, 6), (3, 7))):
                    enq_wq_dma(cq, fq); enq_wq_dma(cq, fk)
                    if cn is not None:
                        enq_xt_tr(cn, 2 * i); enq_xt_tr(cn, 2 * i + 1)
                    enq_qk_f(cq, fq); enq_qk_f(cq, fk)
                if cn is not None:
                    enq_xt_tr(cn, 6); enq_xt_tr(cn, 7)
                for c, half in cps:
                    enq_cproj(c, half)

            def block_pre(c):
                """Mandatory prerequisites for attn(c, 0)."""
                enq_wq_dma(c, 0); enq_wq_dma(c, 4)
                enq_qk_f(c, 0); enq_qk_f(c, 4)
                enq_v(c)
                require(f"qk{c}_0", f"qk{c}_4", f"v{c}")

            def attn_block(c):
                attn_pair(c, 0)
                require(f"qk{c}_1", f"qk{c}_5")
                attn_pair(c, 1)
                require(f"qk{c}_2", f"qk{c}_6")
                attn_pair(c, 2)
                require(f"qk{c}_3", f"qk{c}_7")
                attn_pair(c, 3)

            # chunk-0 prerequisites run inline (startup ramp)
            init_xt(0, xin_pre=xin0)
            for c in (1, 2, 3):
                init_xt(c)
            for kc in range(KT):
                enq_xt_tr(0, kc)
            wq_tiles[(0, 0)] = wq00
            wq_tiles[(0, 4)] = wq04
            wq_dma_enqueued.add((0, 0)); wq_dma_enqueued.add((0, 4))
            require("xt0")
            load_v_consts()     # wv rides the sync queue behind x tiles
            block_pre(0)
            load_a_consts()
            enq_block_fillers(0, 1, [])
            attn_block(0)
            require("xt1")
            load_c_consts()     # wp on sync queue, needed mid-block-1

            block_pre(1)
            enq_block_fillers(1, 2, [(0, 0), (0, 1)])
            attn_block(1)
            require("xt2")

            block_pre(2)
            enq_block_fillers(2, 3, [(1, 0), (1, 1)])
            attn_block(2)
            require("xt3")

            block_pre(3)
            enq_block_fillers(3, None, [(2, 0), (2, 1)])
            attn_block(3)

            enq_cproj(3, 0)
            enq_cproj(3, 1)
            flush_all()

    nc.compile()
    return nc


_NC_CACHE = []


def _get_nc():
    if not _NC_CACHE:
        _NC_CACHE.append(build_nc())
    return _NC_CACHE[0]


def _host_consts():
    ident = np.eye(128, dtype=np.float32).astype(BF)
    kk = np.arange(128, dtype=np.int64)[:, None]
    jj = np.arange(128, dtype=np.int64)[None, :]
    masks = np.where(jj < kk, NEG, 0.0).astype(np.float32).astype(BF)
    return ident, masks


def _make_in_maps(x, W_attn, b_attn, W_proj, b_proj):
    ident, masks = _host_consts()
    in_maps = []
    for core in range(8):
        b, hg = core // 2, core % 2
        sl = slice(hg * 512, (hg + 1) * 512)
        w_q = W_attn[:, 0:1024][:, sl]
        w_k = W_attn[:, 1024:2048][:, sl]
        w_v = W_attn[:, 2048:3072][:, sl]
        in_maps.append({
            "x_l": np.ascontiguousarray(x[b]).astype(BF),
            "w_qk": np.ascontiguousarray(
                np.concatenate([w_q, w_k], axis=1).reshape(8, 128, 8, 128)
                .transpose(2, 1, 0, 3).reshape(1024, 1024)).astype(BF),
            "w_v": np.ascontiguousarray(
                w_v.reshape(8, 128, 512).transpose(1, 0, 2)
                .reshape(128, 4096)).astype(BF),
            "w_p": np.ascontiguousarray(
                W_proj[sl, :].reshape(4, 128, 2, 512).transpose(1, 0, 2, 3)
                .reshape(128, 4096)).astype(BF),
            "b_qk": np.ascontiguousarray(
                np.concatenate([b_attn[0:1024][sl], b_attn[1024:2048][sl]])
                .reshape(8, 128).T),
            "b_v": np.ascontiguousarray(b_attn[2048:3072][sl]),
            "b_o": (b_proj if hg == 0
                    else np.zeros_like(b_proj)).astype(np.float32),
            "ident": ident,
            "masks": masks,
        })
    return in_maps


def _run(inputs, trace=False):
    x = np.asarray(inputs["x"], dtype=np.float32)
    W_attn = np.asarray(inputs["W_attn"], dtype=np.float32)
    b_attn = np.asarray(inputs["b_attn"], dtype=np.float32)
    W_proj = np.asarray(inputs["W_proj"], dtype=np.float32)
    b_proj = np.asarray(inputs["b_proj"], dtype=np.float32)

    nc = _get_nc()
    in_maps = _make_in_maps(x, W_attn, b_attn, W_proj, b_proj)
    res = run_bass_kernel_spmd(nc, in_maps, core_ids=list(range(8)),
                               trace=trace)
    out = np.empty((B, T, C), dtype=np.float32)
    for b in range(B):
        out[b] = (res.results[2 * b]["out_p"].astype(np.float32)
                  + res.results[2 * b + 1]["out_p"].astype(np.float32))
    return out, res


def kernel(**inputs) -> np.ndarray:
    out, _ = _run(inputs, trace=False)
    return out
